# revision 5
# baseline (speedup 1.0000x reference)
"""v7: fp8(e3m4), 32-col-padded samples, shared FWL ldweights per 4 samples.

Per core (S=4096): xt [128, S*32] fp8 (27 data + 5 zero pad per sample).
Per 4-sample group: one full-array 128-col LDWEIGHTS (FWL) + 4 tile_position
quadrant matmuls with ldweights=False streaming 27 cols each (concurrent
col-groups). PSUM f32 -> SBUF bf16 copy, one contiguous [128, 3456] output
DMA per 512-sample chunk. Host packs fp8, unpacks padded gram, overwrites
diagonal + dense passthrough with exact f32.
"""

import os
import sys

import numpy as np

for _p in (
    "/root/.axon_site",
    "/root/.axon_site/_ro/trn_rl_repo",
    "/opt/trn_rl_repo",
):
    if os.path.isdir(_p) and _p not in sys.path:
        sys.path.append(_p)

import ml_dtypes

import concourse.bacc as bacc
import concourse.mybir as mybir
import concourse.tile as tile

NF = 27
SW = 32  # padded sample width in xt
D = 128
B = 32768
NCORES = 8
S = B // NCORES

F32 = mybir.dt.float32
BF16 = mybir.dt.bfloat16
FP8 = mybir.dt.float8e3
NP_FP8 = ml_dtypes.float8_e3m4
NP_BF16 = ml_dtypes.bfloat16

TOFF = np.concatenate([[0], np.cumsum(NF - np.arange(NF))]).astype(np.int64)
NPAIRS = int(TOFF[NF])
DOUT = D + NPAIRS

JB = 16  # 4-sample groups per psum tile
KB = 8  # psum tiles per chunk
J_TOT = JB * KB  # 128
C_SZ = 4 * J_TOT  # 512 samples per chunk
NCHUNKS = S // C_SZ  # 8
BANK = 4 * JB  # 64 samples per psum tile


def build_nc(s_per_core=S, jb=JB, kb=KB):
    j_tot = jb * kb
    c_sz = 4 * j_tot
    assert s_per_core % c_sz == 0
    nchunks = s_per_core // c_sz
    bank_sz = 4 * jb  # samples per psum tile
    assert kb % 2 == 0

    nc = bacc.Bacc("TRN2", target_bir_lowering=False, debug=False)
    xt = nc.dram_tensor("xt", [D, s_per_core * SW], FP8, kind="ExternalInput")
    gram = nc.dram_tensor(
        "gram", [D, s_per_core * NF // 4], BF16, kind="ExternalOutput"
    )

    with tile.TileContext(nc) as tc:
        with (
            tc.tile_pool(name="xin", bufs=8) as xin_pool,
            tc.tile_pool(name="gbuf", bufs=2) as gbuf_pool,
            tc.tile_pool(name="ps", bufs=8, space="PSUM") as ps_pool,
        ):
            in_engines = [nc.sync, nc.scalar]
            rr = [0]
            for c0 in range(nchunks):
                gbuf = gbuf_pool.tile([128, j_tot * NF], BF16)
                for b2 in range(kb // 2):
                    s_base = c0 * c_sz + b2 * 2 * bank_sz
                    xin = xin_pool.tile([D, 2 * bank_sz * SW], FP8)
                    eng = in_engines[rr[0] % 2]
                    rr[0] += 1
                    eng.dma_start(
                        out=xin[:],
                        in_=xt[:, s_base * SW : (s_base + 2 * bank_sz) * SW],
                    )
                    for bh in range(2):
                        b = b2 * 2 + bh
                        ps = ps_pool.tile([128, jb * NF], F32)
                        for t in range(jb):
                            base = (bh * bank_sz + 4 * t) * SW
                            nc.tensor.ldweights(weights=xin[:, base : base + 128])
                            for g in range(4):
                                mm = nc.tensor.matmul(
                                    ps[
                                        32 * g : 32 * g + 32,
                                        t * NF : (t + 1) * NF,
                                    ],
                                    xin[:, base + 32 * g : base + 32 * g + 32],
                                    xin[:, base + 32 * g : base + 32 * g + NF],
                                    start=True,
                                    stop=True,
                                    tile_position=(0, 32 * g),
                                )
                                mm.ins.ldweights = False
                        nc.vector.tensor_copy(
                            gbuf[:, b * jb * NF : (b + 1) * jb * NF], ps[:]
                        )
                nc.gpsimd.dma_start(
                    out=gram[:, c0 * j_tot * NF : (c0 + 1) * j_tot * NF],
                    in_=gbuf[:],
                )
    nc.finalize()
    return nc


def host_pack_inputs(dense_features, sparse_features):
    bsz = dense_features.shape[0]
    xt = np.zeros((D, bsz, SW), dtype=NP_FP8)
    xt[:, :, 0] = dense_features.T.astype(NP_FP8)
    xt[:, :, 1:NF] = sparse_features.transpose(2, 0, 1).astype(NP_FP8)
    return xt


def host_core_input(xt, c, s_per_core=S):
    return np.ascontiguousarray(
        xt[:, c * s_per_core : (c + 1) * s_per_core, :]
    ).reshape(D, s_per_core * SW)


_TRIU_R, _TRIU_C = np.triu_indices(NF, k=0)


def host_unpack_output(dense_features, sparse_features, gram_cores):
    bsz = dense_features.shape[0]
    out = np.empty((bsz, DOUT), dtype=np.float32)
    out[:, :D] = dense_features

    # gram_cores: [128, S*27/4] bf16 per core.
    # partition 32g+n, col c*3456 + b*432 + t*27 + m  <->  sample
    # c*512 + b*64 + 4t + g, entry (n, m).
    gram = np.empty((bsz, NF, NF), dtype=np.float32)
    for ci, gp in enumerate(gram_cores):
        v = np.asarray(gp).reshape(4, 32, NCHUNKS, KB, JB, NF)
        # index [g, n, c, b, t, m] -> sample c*512 + b*64 + 4t + g
        v = v.transpose(2, 3, 4, 0, 1, 5)  # [c, b, t, g, 32, m]
        v = v.reshape(S, 32, NF)[:, :NF, :].astype(np.float32)
        gram[ci * S : (ci + 1) * S] = v
    out[:, D:] = gram[:, _TRIU_R, _TRIU_C]

    # exact diagonal (||feature||^2) from the f32 inputs
    dsq = np.einsum("bd,bd->b", dense_features, dense_features)
    ssq = np.einsum("bnd,bnd->bn", sparse_features, sparse_features)
    for n in range(NF):
        col = D + int(TOFF[n])
        out[:, col] = dsq if n == 0 else ssq[:, n - 1]
    return out


_NC_CACHE = {}


def _get_nc():
    key = (S,)
    if key not in _NC_CACHE:
        _NC_CACHE[key] = build_nc(S)
    return _NC_CACHE[key]


def kernel(dense_features, sparse_features):
    from concourse.bass_utils import run_bass_kernel_spmd

    dense_features = np.asarray(dense_features, dtype=np.float32)
    sparse_features = np.asarray(sparse_features, dtype=np.float32)
    xt = host_pack_inputs(dense_features, sparse_features)
    in_maps = [{"xt": host_core_input(xt, c)} for c in range(NCORES)]
    nc = _get_nc()
    res = run_bass_kernel_spmd(nc, in_maps, core_ids=list(range(NCORES)))
    gram_cores = [r["gram"] for r in res.results]
    return host_unpack_output(dense_features, sparse_features, gram_cores)


# revision 8
# speedup vs baseline: 1.7543x; 1.7543x over previous
"""v8: raw-bass pipeline, fp8(e3m4) in / bf16 out, 27-col weight loads,
one semaphore inc per PSUM tile (64 MMs) instead of per-MM.

Per core (S=4096): xt [128, S*27] fp8. Per sample: LDWEIGHTS(27 cols) +
MATMUL(27 streams) rotating over the 4 PE column-quadrants
(tile_position=(0,32g)), PSUM f32 -> SBUF bf16 copy per 64-sample tile,
one contiguous [128, 3456] output DMA per 512-sample chunk. Host packs
fp8, unpacks the padded gram, overwrites diagonal + dense passthrough
with exact f32.

Manual semaphore protocol (all cleared at end of program):
  s_in_e/s_in_o: input-window DMA completions (sync/scalar queues)
  s_mm:  last MM of each PSUM tile        (PE -> DVE copy ready)
  s_cp:  copy completions                 (DVE -> out DMA / PSUM reuse)
  s_out: output DMA completions           (gbuf reuse)
"""

import os
import sys

import numpy as np

for _p in (
    "/root/.axon_site",
    "/root/.axon_site/_ro/trn_rl_repo",
    "/opt/trn_rl_repo",
):
    if os.path.isdir(_p) and _p not in sys.path:
        sys.path.append(_p)

import ml_dtypes

import concourse.bacc as bacc
import concourse.mybir as mybir

NF = 27
D = 128
B = 32768
NCORES = 8
S = B // NCORES

F32 = mybir.dt.float32
BF16 = mybir.dt.bfloat16
FP8 = mybir.dt.float8e3
NP_FP8 = ml_dtypes.float8_e3m4

TOFF = np.concatenate([[0], np.cumsum(NF - np.arange(NF))]).astype(np.int64)
NPAIRS = int(TOFF[NF])
DOUT = D + NPAIRS

JB = 16  # samples per quadrant per psum tile
KB = 8  # psum tiles per chunk
C_SZ = 4 * JB * KB  # 512 samples per chunk
NCHUNKS = S // C_SZ  # 8
WIN = 128  # samples per input DMA window
NBUF = 8  # xin double buffering depth


def build_nc(s_per_core=S):
    nc = bacc.Bacc("TRN2", target_bir_lowering=False, debug=False)
    xt = nc.dram_tensor("xt", [D, s_per_core * NF], FP8, kind="ExternalInput")
    gram = nc.dram_tensor(
        "gram", [D, s_per_core * NF // 4], BF16, kind="ExternalOutput"
    )

    n_win = s_per_core // WIN  # 32
    n_tile = s_per_core // 64  # 64
    n_ch = s_per_core // C_SZ  # 8

    xin = [
        nc.alloc_sbuf_tensor(f"xin{i}", [D, WIN * NF], FP8) for i in range(NBUF)
    ]
    gbuf = [
        nc.alloc_sbuf_tensor(f"gbuf{i}", [D, C_SZ * NF // 4], BF16)
        for i in range(2)
    ]
    ps = [
        nc.place_psum_tensor(f"ps{i}", [128, JB * NF], F32, bank=i)
        for i in range(8)
    ]

    s_in = [nc.alloc_semaphore("s_in_e"), nc.alloc_semaphore("s_in_o")]
    s_mm = nc.alloc_semaphore("s_mm")
    s_cp = nc.alloc_semaphore("s_cp")
    s_out = nc.alloc_semaphore("s_out")

    in_eng = [nc.sync, nc.scalar]

    # input DMAs (alternating HWDGE queues); buffer reuse gated on the MMs
    # of the window that previously held the buffer
    for w in range(n_win):
        d = in_eng[w % 2].dma_start(
            out=xin[w % NBUF][:, :],
            in_=xt[:, w * WIN * NF : (w + 1) * WIN * NF],
        )
        if w >= NBUF:
            d._wait_ge(s_mm, 2 * (w - NBUF) + 2)
        d.then_inc(s_in[w % 2], 16)

    # PE: per sample LDWEIGHTS+MATMUL rotating quadrants
    for t in range(n_tile):
        w, bh = divmod(t, 2)
        xb = xin[w % NBUF]
        pst = ps[t % 8]
        if bh == 0:
            nc.tensor.wait_ge(s_in[w % 2], 16 * (w // 2 + 1))
        if t >= 8:
            nc.tensor.wait_ge(s_cp, t - 7)
        mm = None
        for jbi in range(JB):
            for g in range(4):
                loc = (bh * 64 + g * JB + jbi) * NF
                mm = nc.tensor.matmul(
                    pst[32 * g : 32 * g + NF, jbi * NF : (jbi + 1) * NF],
                    xb[:, loc : loc + NF],
                    xb[:, loc : loc + NF],
                    start=True,
                    stop=True,
                    tile_position=(0, 32 * g),
                )
        mm.then_inc(s_mm)

    # DVE: PSUM -> SBUF bf16 copies
    for t in range(n_tile):
        c, slot = divmod(t, KB)
        if slot == 0 and c >= 2:
            nc.vector.wait_ge(s_out, 16 * (c - 1))
        cp = nc.vector.tensor_copy(
            gbuf[c % 2][:, slot * JB * NF : (slot + 1) * JB * NF],
            ps[t % 8][:, :],
        )
        cp._wait_ge(s_mm, t + 1)
        cp.then_inc(s_cp)

    # gpsimd: output DMAs, one per chunk
    cw = C_SZ * NF // 4  # 3456 columns per chunk
    for c in range(n_ch):
        d = nc.gpsimd.dma_start(
            out=gram[:, c * cw : (c + 1) * cw], in_=gbuf[c % 2][:, :]
        )
        d._wait_ge(s_cp, KB * (c + 1))
        d.then_inc(s_out, 16)

    # leave all semaphores at 0 for the next execution
    nc.sync.wait_ge(s_out, 16 * n_ch)
    for sm in (s_in[0], s_in[1], s_mm, s_cp, s_out):
        nc.sync.sem_clear(sm)

    nc.finalize()
    return nc


def host_pack_inputs(dense_features, sparse_features):
    bsz = dense_features.shape[0]
    xt = np.empty((D, bsz, NF), dtype=NP_FP8)
    xt[:, :, 0] = dense_features.T.astype(NP_FP8)
    xt[:, :, 1:] = sparse_features.transpose(2, 0, 1).astype(NP_FP8)
    return xt


def host_core_input(xt, c, s_per_core=S):
    return np.ascontiguousarray(
        xt[:, c * s_per_core : (c + 1) * s_per_core, :]
    ).reshape(D, s_per_core * NF)


_TRIU_R, _TRIU_C = np.triu_indices(NF, k=0)


def host_unpack_output(dense_features, sparse_features, gram_cores):
    bsz = dense_features.shape[0]
    out = np.empty((bsz, DOUT), dtype=np.float32)
    out[:, :D] = dense_features

    # gram_cores: [128, S*27/4] bf16 per core.
    # partition 32g+n, col c*3456 + b*432 + j*27 + m  <->  sample
    # c*512 + b*64 + g*16 + j, entry (n, m).
    gram = np.empty((bsz, NF, NF), dtype=np.float32)
    for ci, gp in enumerate(gram_cores):
        v = np.asarray(gp).reshape(4, 32, NCHUNKS, KB, JB, NF)
        v = v.transpose(2, 3, 0, 4, 1, 5)  # [c, b, g, j, 32, m]
        v = v.reshape(S, 32, NF)[:, :NF, :].astype(np.float32)
        gram[ci * S : (ci + 1) * S] = v
    out[:, D:] = gram[:, _TRIU_R, _TRIU_C]

    # exact diagonal (||feature||^2) from the f32 inputs
    dsq = np.einsum("bd,bd->b", dense_features, dense_features)
    ssq = np.einsum("bnd,bnd->bn", sparse_features, sparse_features)
    for n in range(NF):
        col = D + int(TOFF[n])
        out[:, col] = dsq if n == 0 else ssq[:, n - 1]
    return out


_NC_CACHE = {}


def _get_nc():
    key = (S,)
    if key not in _NC_CACHE:
        _NC_CACHE[key] = build_nc(S)
    return _NC_CACHE[key]


def kernel(dense_features, sparse_features):
    from concourse.bass_utils import run_bass_kernel_spmd

    dense_features = np.asarray(dense_features, dtype=np.float32)
    sparse_features = np.asarray(sparse_features, dtype=np.float32)
    xt = host_pack_inputs(dense_features, sparse_features)
    in_maps = [{"xt": host_core_input(xt, c)} for c in range(NCORES)]
    nc = _get_nc()
    res = run_bass_kernel_spmd(nc, in_maps, core_ids=list(range(NCORES)))
    gram_cores = [r["gram"] for r in res.results]
    return host_unpack_output(dense_features, sparse_features, gram_cores)
